# revision 1
# baseline (speedup 1.0000x reference)
"""Trainium2 Bass kernel for GroupedKAAttention.

Math (per batch row b of B=4096, fp32 reference):
  xg[b,g,:]  = x[b, g*64:(g+1)*64]                      (G=64 groups, D=64)
  h[b,g,:]   = silu(xg[b,g,:] @ W1[g] + b1[g])          (H=512)
  f[b,g,:]   = h[b,g,:] @ W2[g] + b2[g]                 (P=64 patches)
  h2[b,p,:]  = silu(f[b,:,p] @ Wg1 + bg1)               (contract groups)
  o[b,p,:]   = h2[b,p,:] @ Wg2 + bg2                    (E=16 heads)
  attn[b]    = sum_{p,e} o_q * o_k ;  out = softmax(attn over b)

Distribution: data-parallel over batch, 512 rows per core, weights
replicated.  Activations live feature-on-partition / batch-on-free.
Matmuls run in fp32r (~1.6e-4 rel err, full PE speed at N=512; fp32
PSUM accumulate).  The grouped-stage bias is baked in as a 65th
contraction row; SiLU runs on the scalar engine while draining PSUM
(that engine is the critical path: ~67M evals/core).  The (g,p)
transpose between grouped and global stages is a DRAM bounce with
strided DMA access patterns.  The global stage row-tiles patch pairs
(fp32r forbids column tile_position).  Per-core output is 512 attention
logits; softmax over the full 4096 batch is applied on host.
"""

import numpy as np

B = 4096
TOTAL_DIM = 4096
G = 64            # groups
D = 64            # group size
H = 512           # hidden
P = 64            # patches
E = 16            # heads
NCORES = 8
BC = B // NCORES  # 512 batch rows per core
NPAIR = P // 2    # 32 patch pairs (global stage)


def _build_nc():
    from contextlib import ExitStack
    import concourse.bass as bass
    import concourse.tile as tile
    import concourse.mybir as mybir
    from concourse import bacc

    dt = mybir.dt
    fr = dt.float32r
    f32 = dt.float32
    AF = mybir.ActivationFunctionType

    nc = bacc.Bacc(
        "TRN2",
        target_bir_lowering=False,
        debug=False,
        enable_asserts=False,
        num_devices=NCORES,
    )

    ins = {}
    def din(name, shape, dty):
        ins[name] = nc.dram_tensor(name, shape, dty, kind="ExternalInput").ap()
        return ins[name]

    xq = din("xq", [G * (D + 1), BC], fr)      # rows g*65+d (d<64: x^T), row 64: ones
    xk = din("xk", [G * (D + 1), BC], fr)
    w1q = din("w1q", [G * (D + 1), H], fr)     # rows g*65+d: W1[g,d,:], row 64: b1[g]
    w1k = din("w1k", [G * (D + 1), H], fr)
    w2q = din("w2q", [G * 128, 4 * 64], fr)    # group g rows: [r, hc*64+p] = W2[g, hc*128+r, p]
    w2k = din("w2k", [G * 128, 4 * 64], fr)
    wg1 = din("wg1", [128, H], fr)             # Wg1 [64,512] duplicated on both partition halves
    wg2 = din("wg2", [128, 4 * 32], fr)        # [r, hc*32+e] = Wg2[hc*128+r, e] (e<16, else 0)
    b2q = din("b2q", [64, G], f32)             # col g = b2[g]
    b2k = din("b2k", [64, G], f32)
    bg1p = din("bg1p", [128, 4], f32)          # col hc = bg1[hc*128:(hc+1)*128]
    bg2r = din("bg2r", [128, 1], f32)          # 4x [bg2(16); zeros(16)] along partitions
    ones128 = din("ones128", [128, 1], fr)

    out = nc.dram_tensor("out", [1, BC], f32, kind="ExternalOutput").ap()

    with tile.TileContext(nc) as tc:
        with ExitStack() as ctx:
            ep = ctx.enter_context
            px = ep(tc.tile_pool(name="px", bufs=6))          # x tiles [65,BC]
            pw1 = ep(tc.tile_pool(name="pw1", bufs=6))        # W1 tiles [65,H]
            pw2 = ep(tc.tile_pool(name="pw2", bufs=4))        # W2 group tiles [128,256]
            phs = ep(tc.tile_pool(name="phs", bufs=4))        # silu'd h [128,1024]
            pfv = ep(tc.tile_pool(name="pfv", bufs=4))        # f group tiles [64,BC]
            pu = ep(tc.tile_pool(name="pu", bufs=6))          # U tiles [128,BC]
            ph2 = ep(tc.tile_pool(name="ph2", bufs=10))       # silu'd h2 [128,1024]
            pbig = ep(tc.tile_pool(name="pbig", bufs=1))      # qs/ks/prod [128,8*BC]
            pmisc = ep(tc.tile_pool(name="pmisc", bufs=2))
            pconst = ep(tc.tile_pool(name="pconst", bufs=1))
            # PSUM: psh 3 x 2 banks + psv 2 x 1 bank = 8 banks
            psh = ep(tc.tile_pool(name="psh", bufs=3, space="PSUM"))
            psv = ep(tc.tile_pool(name="psv", bufs=2, space="PSUM"))
            pdram = ep(tc.tile_pool(name="pdram", bufs=1, space="DRAM"))

            def const_tile(src_ap, shape, dty, name):
                t = pconst.tile(shape, dty, name=name, tag=name)
                nc.sync.dma_start(t[:, :], src_ap)
                return t

            wg1_s = const_tile(wg1, [128, H], fr, "wg1s")
            wg2_s = const_tile(wg2, [128, 4 * 32], fr, "wg2s")
            b2q_s = const_tile(b2q, [64, G], f32, "b2qs")
            b2k_s = const_tile(b2k, [64, G], f32, "b2ks")
            bg1_s = const_tile(bg1p, [128, 4], f32, "bg1s")
            bg2_s = const_tile(bg2r, [128, 1], f32, "bg2s")
            one_s = const_tile(ones128, [128, 1], fr, "ones")

            f_dram = {
                "q": pdram.tile([G * P, BC], fr, name="fq", tag="fq"),
                "k": pdram.tile([G * P, BC], fr, name="fk", tag="fk"),
            }
            stream_in = {"q": (xq, w1q, w2q, b2q_s), "k": (xk, w1k, w2k, b2k_s)}

            # ================= grouped stage =================
            def grouped(s):
                x_d, w1_d, w2_d, b2_s = stream_in[s]
                fd = f_dram[s]
                for g in range(G):
                    x_t = px.tile([D + 1, BC], fr, tag="x")
                    nc.sync.dma_start(x_t[:, :], x_d[g * 65:(g + 1) * 65, :])
                    w1_t = pw1.tile([D + 1, H], fr, tag="w1")
                    nc.sync.dma_start(w1_t[:, :], w1_d[g * 65:(g + 1) * 65, :])
                    w2_t = pw2.tile([128, 4 * 64], fr, tag="w2")
                    nc.sync.dma_start(w2_t[:, :], w2_d[g * 128:(g + 1) * 128, :])
                    v_ps = psv.tile([64, BC], f32, tag="vps")
                    for t in range(2):       # two [128,1024] PSUM tiles = 4 h-chunks
                        hp = psh.tile([128, 1024], f32, tag="hps")
                        for u in range(2):
                            hc = 2 * t + u
                            nc.tensor.matmul(
                                hp[:, u * 512:(u + 1) * 512],
                                w1_t[:, hc * 128:(hc + 1) * 128],
                                x_t[:, :],
                                start=True, stop=True,
                            )
                        hs_t = phs.tile([128, 1024], fr, tag="hs")
                        nc.scalar.activation(hs_t[:, :], hp[:, :], AF.Silu)
                        for u in range(2):   # GEMM2 accumulation
                            hc = 2 * t + u
                            nc.tensor.matmul(
                                v_ps[:, :],
                                w2_t[:, hc * 64:(hc + 1) * 64],
                                hs_t[:, u * 512:(u + 1) * 512],
                                start=(hc == 0), stop=(hc == 3),
                            )
                    fv = pfv.tile([64, BC], fr, tag="fv")
                    nc.vector.tensor_scalar_add(fv[:, :], v_ps[:, :], b2_s[:, g:g + 1])
                    nc.sync.dma_start(fd[g * 64:(g + 1) * 64, :], fv[:, :])

            # ================= global stage =================
            def global_stream(s, big):
                fd3 = f_dram[s].rearrange("(g p) b -> p g b", p=P)
                for j in range(NPAIR):       # patch pair (2j, 2j+1)
                    u_t = pu.tile([128, BC], fr, tag="u")
                    nc.sync.dma_start(u_t[:, :], fd3[2 * j:2 * j + 2])
                    h2s = []
                    for hc in range(4):
                        h2p = psh.tile([128, 1024], f32, tag="hps")
                        for dp in range(2):
                            nc.tensor.matmul(
                                h2p[:, dp * 512:(dp + 1) * 512],
                                wg1_s[dp * 64:(dp + 1) * 64, hc * 128:(hc + 1) * 128],
                                u_t[dp * 64:(dp + 1) * 64, :],
                                start=True, stop=True,
                                tile_position=(dp * 64, 0),
                            )
                        t = ph2.tile([128, 1024], fr, tag="h2s")
                        nc.scalar.activation(t[:, :], h2p[:, :], AF.Silu,
                                             bias=bg1_s[:, hc:hc + 1])
                        h2s.append(t)
                    for dp in range(2):      # head GEMM per patch (M=32, top 16 real)
                        p_ = 2 * j + dp
                        o_ps = psv.tile([32, BC], f32, tag="vps")
                        for hc in range(4):
                            nc.tensor.matmul(
                                o_ps[:, :],
                                wg2_s[:, hc * 32:(hc + 1) * 32],
                                h2s[hc][:, dp * 512:(dp + 1) * 512],
                                start=(hc == 0), stop=(hc == 3),
                            )
                        # drain into big [128, 16*BC]: partition 32*(p%4), col-block p//4
                        pr, pcb = 32 * (p_ % 4), (p_ // 4) * BC
                        nc.vector.tensor_scalar_add(
                            big[pr:pr + 32, pcb:pcb + BC], o_ps[:, :],
                            bg2_s[pr:pr + 32, 0:1])

            grouped("q")
            grouped("k")

            qs_big = pbig.tile([128, 16 * BC], f32, tag="qsbig")
            ks_big = pbig.tile([128, 16 * BC], f32, tag="ksbig")
            global_stream("q", qs_big)
            global_stream("k", ks_big)

            # ============ dot product + logits ============
            prod = ks_big   # in-place q*k
            nc.vector.tensor_mul(prod[:, :], qs_big[:, :], ks_big[:, :])
            red = pmisc.tile([128, BC], fr, tag="red")
            with nc.allow_low_precision(reason="fp32r reduce of 8 fp32 blocks"):
                nc.vector.tensor_reduce(
                    red[:, :],
                    prod[:, :].rearrange("a (c b) -> a b c", b=BC),
                    axis=mybir.AxisListType.X,
                    op=mybir.AluOpType.add,
                )
            at_ps = psv.tile([1, BC], f32, tag="vps")
            nc.tensor.matmul(at_ps[0:1, :], one_s[:, 0:1], red[:, :],
                             start=True, stop=True)
            at_s = pmisc.tile([1, BC], f32, tag="at")
            nc.vector.tensor_copy(at_s[0:1, :], at_ps[0:1, :])
            nc.sync.dma_start(out[0:1, :], at_s[0:1, :])

    nc.compile()
    return nc


_NC_CACHE = None


def _get_nc():
    global _NC_CACHE
    if _NC_CACHE is None:
        _NC_CACHE = _build_nc()
    return _NC_CACHE


def _prep_inputs(q, k, W1q, b1q, W2q, b2q, W1k, b1k, W2k, b2k, Wg1, bg1, Wg2, bg2):
    f32c = lambda a: np.ascontiguousarray(a, dtype=np.float32)

    def pack_x(x):  # [B, 4096] -> per-core [G*65, BC] with ones row
        shards = []
        for c in range(NCORES):
            xs = x[c * BC:(c + 1) * BC, :]
            xt = np.empty((G, D + 1, BC), dtype=np.float32)
            xt[:, :D, :] = xs.T.reshape(G, D, BC)
            xt[:, D, :] = 1.0
            shards.append(f32c(xt.reshape(G * (D + 1), BC)))
        return shards

    def pack_w1(W1, b1):
        w = np.concatenate([np.asarray(W1, np.float32),
                            np.asarray(b1, np.float32)[:, None, :]], axis=1)
        return f32c(w.reshape(G * (D + 1), H))

    def pack_w2(W2):  # [G, 512, 64] -> [G*128, 4*64]
        w = np.asarray(W2, np.float32).reshape(G, 4, 128, 64)   # [g, hc, r, p]
        w = w.transpose(0, 2, 1, 3)                             # [g, r, hc, p]
        return f32c(w.reshape(G * 128, 4 * 64))

    xq_s = pack_x(q)
    xk_s = pack_x(k)
    w1q_p = pack_w1(W1q, b1q)
    w1k_p = pack_w1(W1k, b1k)
    w2q_p = pack_w2(W2q)
    w2k_p = pack_w2(W2k)
    b2q_p = f32c(np.asarray(b2q, np.float32).T)                 # [64(P), G]
    b2k_p = f32c(np.asarray(b2k, np.float32).T)

    wg1_p = f32c(np.concatenate([Wg1, Wg1], axis=0))            # [128, 512]
    wg2_p = np.zeros((128, 4, 32), dtype=np.float32)
    wg2_p[:, :, :E] = np.asarray(Wg2, np.float32).reshape(4, 128, E).transpose(1, 0, 2)
    wg2_p = f32c(wg2_p.reshape(128, 4 * 32))                    # [r, hc*32+e]
    bg1_p = f32c(np.asarray(bg1, np.float32).reshape(4, 128).T)  # [128, 4]
    bg2_p = np.zeros((4, 32), dtype=np.float32)
    bg2_p[:, :E] = np.asarray(bg2, np.float32)
    bg2_p = f32c(bg2_p.reshape(128, 1))
    ones_p = np.ones((128, 1), dtype=np.float32)

    in_maps = []
    for c in range(NCORES):
        in_maps.append({
            "xq": xq_s[c], "xk": xk_s[c],
            "w1q": w1q_p, "w1k": w1k_p,
            "w2q": w2q_p, "w2k": w2k_p,
            "wg1": wg1_p, "wg2": wg2_p,
            "b2q": b2q_p, "b2k": b2k_p,
            "bg1p": bg1_p, "bg2r": bg2_p, "ones128": ones_p,
        })
    return in_maps


def kernel(q, k, W1q, b1q, W2q, b2q, W1k, b1k, W2k, b2k, Wg1, bg1, Wg2, bg2,
           _trace=False, _tracedir=None):
    from concourse.bass_utils import run_bass_kernel_spmd

    in_maps = _prep_inputs(q, k, W1q, b1q, W2q, b2q, W1k, b1k, W2k, b2k,
                           Wg1, bg1, Wg2, bg2)
    nc = _get_nc()
    kw = {}
    if _trace:
        kw = dict(trace=True, tmpdir=_tracedir)
    res = run_bass_kernel_spmd(nc, in_maps, core_ids=list(range(NCORES)), **kw)
    logits = np.concatenate([res.results[c]["out"].reshape(BC)
                             for c in range(NCORES)]).astype(np.float64)
    m = logits.max()
    e = np.exp(logits - m)
    sm = (e / e.sum()).astype(np.float32)
    if _trace:
        kernel._last_trace = res
    return sm



# revision 2
# speedup vs baseline: 5.4653x; 5.4653x over previous
"""Trainium2 Bass kernel for GroupedKAAttention.

Math (per batch row b of B=4096, fp32 reference):
  xg[b,g,:]  = x[b, g*64:(g+1)*64]                      (G=64 groups, D=64)
  h[b,g,:]   = silu(xg[b,g,:] @ W1[g] + b1[g])          (H=512)
  f[b,g,:]   = h[b,g,:] @ W2[g] + b2[g]                 (P=64 patches)
  h2[b,p,:]  = silu(f[b,:,p] @ Wg1 + bg1)               (contract groups)
  o[b,p,:]   = h2[b,p,:] @ Wg2 + bg2                    (E=16 heads)
  attn[b]    = sum_{p,e} o_q * o_k ;  out = softmax(attn over b)

Distribution: the wall clock is dominated by host->device transfer over
the axon tunnel (~70 MB/s), so the layout minimizes shipped bytes:
  - grouped stage is GROUP-sharded: core c owns groups 8c..8c+7 and runs
    them over the FULL batch, so W1/W2 are sharded (1/8 the bytes) and
    each core receives only its 512 columns of x (no replication);
  - an on-device AllToAll (fp16, 4.2MB/stream over NeuronLink) re-shards
    the intermediate f from group-sharded to batch-sharded, landing in
    the [g*64+p, b_local] layout the global stage consumes;
  - global stage + dot product are batch-parallel (512 rows per core)
    with tiny replicated weights.
All large tensors ship as fp16 (error budget 2e-2; fp16 rounding adds
~1e-3).  Matmuls run fp16 x fp16 with fp32 PSUM accumulation; the
grouped-stage bias+SiLU is fused into scalar-engine activations.
Per-core output is 512 attention logits; softmax over the full 4096
batch is applied on host.
"""

import numpy as np

B = 4096
TOTAL_DIM = 4096
G = 64            # groups
D = 64            # group size
H = 512           # hidden
P = 64            # patches
E = 16            # heads
NCORES = 8
GL = G // NCORES  # 8 local groups per core (stage 1)
BC = B // NCORES  # 512 batch rows per core (stage 2)
NPAIR = P // 2    # 32 patch pairs (global stage)
NBC = B // 512    # 8 batch chunks of 512 in stage 1


def _build_nc():
    from contextlib import ExitStack
    import concourse.bass as bass
    import concourse.tile as tile
    import concourse.mybir as mybir
    from concourse import bacc

    dt = mybir.dt
    fr = dt.float32r
    f32 = dt.float32
    f16 = dt.float16
    AF = mybir.ActivationFunctionType

    nc = bacc.Bacc(
        "TRN2",
        target_bir_lowering=False,
        debug=False,
        enable_asserts=False,
        num_devices=NCORES,
    )

    ins = {}
    def din(name, shape, dty):
        ins[name] = nc.dram_tensor(name, shape, dty, kind="ExternalInput").ap()
        return ins[name]

    # stage-1 inputs, group-sharded (core c holds groups 8c..8c+7)
    xq = din("xq", [GL * D, B], f16)       # row gl*64+d = x[:, c*512+gl*64+d]
    xk = din("xk", [GL * D, B], f16)
    w1q = din("w1q", [GL * D, H], f16)     # rows gl*64+d: W1[g,d,:]
    w1k = din("w1k", [GL * D, H], f16)
    w2q = din("w2q", [GL * 128, 4 * 64], f16)  # group gl rows: [r, hc*64+p] = W2[g, hc*128+r, p]
    w2k = din("w2k", [GL * 128, 4 * 64], f16)
    b1q = din("b1q", [128, GL * 4], f32)   # col gl*4+hc = b1[g, hc*128:(hc+1)*128]
    b1k = din("b1k", [128, GL * 4], f32)
    b2q = din("b2q", [64, GL], f32)        # col gl = b2[g]
    b2k = din("b2k", [64, GL], f32)
    # stage-2 weights, replicated (tiny)
    wg1 = din("wg1", [128, H], f16)        # Wg1 [64,512] duplicated on both partition halves
    wg2 = din("wg2", [128, 4 * 32], f16)   # [r, hc*32+e] = Wg2[hc*128+r, e] (e<16, else 0)
    bg1p = din("bg1p", [128, 4], f32)      # col hc = bg1[hc*128:(hc+1)*128]
    bg2r = din("bg2r", [128, 1], f32)      # 4x [bg2(16); zeros(16)] along partitions
    ones128 = din("ones128", [128, 1], fr)

    out = nc.dram_tensor("out", [1, BC], f32, kind="ExternalOutput").ap()

    with tile.TileContext(nc) as tc:
        with ExitStack() as ctx:
            ep = ctx.enter_context
            px = ep(tc.tile_pool(name="px", bufs=3))          # x group tiles [64,B] f16
            pw1 = ep(tc.tile_pool(name="pw1", bufs=3))        # W1 tiles [64,H] f16
            pw2 = ep(tc.tile_pool(name="pw2", bufs=3))        # W2 group tiles [128,256] f16
            phs = ep(tc.tile_pool(name="phs", bufs=4))        # silu'd h [128,1024] f16
            pfv = ep(tc.tile_pool(name="pfv", bufs=4))        # f tiles [64,512] f16
            pu = ep(tc.tile_pool(name="pu", bufs=6))          # U tiles [128,BC] f16
            ph2 = ep(tc.tile_pool(name="ph2", bufs=10))       # silu'd h2 [128,1024] f16
            pbig = ep(tc.tile_pool(name="pbig", bufs=1))      # qs/ks/prod [128,8*BC] f32
            pmisc = ep(tc.tile_pool(name="pmisc", bufs=2))
            pconst = ep(tc.tile_pool(name="pconst", bufs=1))
            # PSUM: psh 3 x 2 banks + psv 2 x 1 bank = 8 banks
            psh = ep(tc.tile_pool(name="psh", bufs=3, space="PSUM"))
            psv = ep(tc.tile_pool(name="psv", bufs=2, space="PSUM"))
            pdram = ep(tc.tile_pool(name="pdram", bufs=1, space="DRAM"))

            def const_tile(src_ap, shape, dty, name):
                t = pconst.tile(shape, dty, name=name, tag=name)
                nc.sync.dma_start(t[:, :], src_ap)
                return t

            wg1_s = const_tile(wg1, [128, H], f16, "wg1s")
            wg2_s = const_tile(wg2, [128, 4 * 32], f16, "wg2s")
            b1q_s = const_tile(b1q, [128, GL * 4], f32, "b1qs")
            b1k_s = const_tile(b1k, [128, GL * 4], f32, "b1ks")
            b2q_s = const_tile(b2q, [64, GL], f32, "b2qs")
            b2k_s = const_tile(b2k, [64, GL], f32, "b2ks")
            bg1_s = const_tile(bg1p, [128, 4], f32, "bg1s")
            bg2_s = const_tile(bg2r, [128, 1], f32, "bg2s")
            one_s = const_tile(ones128, [128, 1], fr, "ones")

            fsrc = {
                "q": pdram.tile([G * P, BC], f16, name="fsq", tag="fsq"),
                "k": pdram.tile([G * P, BC], f16, name="fsk", tag="fsk"),
            }
            fdst = {
                "q": pdram.tile([G * P, BC], f16, name="fdq", tag="fdq"),
                "k": pdram.tile([G * P, BC], f16, name="fdk", tag="fdk"),
            }
            stream_in = {
                "q": (xq, w1q, w2q, b1q_s, b2q_s),
                "k": (xk, w1k, w2k, b1k_s, b2k_s),
            }

            # ====== stage 1: local groups (8), full batch (4096) ======
            # fsrc rows bc*512 + gl*64 + p; AllToAll swaps chunk bc of core
            # c to chunk c of core bc, giving fdst rows g*64+p, cols local b.
            def grouped(s):
                x_d, w1_d, w2_d, b1_s, b2_s = stream_in[s]
                fd = fsrc[s]
                for gl in range(GL):
                    x_t = px.tile([D, B], f16, tag="x")
                    nc.sync.dma_start(x_t[:, :], x_d[gl * D:(gl + 1) * D, :])
                    w1_t = pw1.tile([D, H], f16, tag="w1")
                    nc.sync.dma_start(w1_t[:, :], w1_d[gl * D:(gl + 1) * D, :])
                    w2_t = pw2.tile([128, 4 * 64], f16, tag="w2")
                    nc.sync.dma_start(w2_t[:, :], w2_d[gl * 128:(gl + 1) * 128, :])
                    for bc in range(NBC):
                        hs_t = phs.tile([128, 2048], f16, tag="hs")
                        for t in range(2):   # two [128,1024] PSUM tiles = 4 h-chunks
                            hp = psh.tile([128, 1024], f32, tag="hps")
                            for u in range(2):
                                hc = 2 * t + u
                                nc.tensor.matmul(
                                    hp[:, u * 512:(u + 1) * 512],
                                    w1_t[:, hc * 128:(hc + 1) * 128],
                                    x_t[:, bc * 512:(bc + 1) * 512],
                                    start=True, stop=True,
                                )
                                nc.scalar.activation(
                                    hs_t[:, hc * 512:(hc + 1) * 512],
                                    hp[:, u * 512:(u + 1) * 512],
                                    AF.Silu,
                                    bias=b1_s[:, gl * 4 + hc:gl * 4 + hc + 1],
                                )
                        v_ps = psv.tile([64, 512], f32, tag="vps")
                        for hc in range(4):   # GEMM2 accumulation
                            nc.tensor.matmul(
                                v_ps[:, :],
                                w2_t[:, hc * 64:(hc + 1) * 64],
                                hs_t[:, hc * 512:(hc + 1) * 512],
                                start=(hc == 0), stop=(hc == 3),
                            )
                        fv = pfv.tile([64, 512], f16, tag="fv")
                        nc.vector.tensor_scalar_add(fv[:, :], v_ps[:, :],
                                                    b2_s[:, gl:gl + 1])
                        nc.sync.dma_start(
                            fd[bc * 512 + gl * 64:bc * 512 + (gl + 1) * 64, :],
                            fv[:, :])

            def exchange(s):
                nc.gpsimd.collective_compute(
                    "AllToAll",
                    mybir.AluOpType.bypass,
                    replica_groups=[list(range(NCORES))],
                    ins=[fsrc[s][:, :]],
                    outs=[fdst[s][:, :]],
                )

            # ====== stage 2: all groups, local batch (512) ======
            def global_stream(s, big):
                fd3 = fdst[s].rearrange("(g p) b -> p g b", p=P)
                for j in range(NPAIR):       # patch pair (2j, 2j+1)
                    u_t = pu.tile([128, BC], f16, tag="u")
                    nc.sync.dma_start(u_t[:, :], fd3[2 * j:2 * j + 2])
                    h2s = []
                    for hc in range(4):
                        h2p = psh.tile([128, 1024], f32, tag="hps")
                        for dp in range(2):
                            nc.tensor.matmul(
                                h2p[:, dp * 512:(dp + 1) * 512],
                                wg1_s[dp * 64:(dp + 1) * 64, hc * 128:(hc + 1) * 128],
                                u_t[dp * 64:(dp + 1) * 64, :],
                                start=True, stop=True,
                                tile_position=(dp * 64, 0),
                            )
                        t = ph2.tile([128, 1024], f16, tag="h2s")
                        nc.scalar.activation(t[:, :], h2p[:, :], AF.Silu,
                                             bias=bg1_s[:, hc:hc + 1])
                        h2s.append(t)
                    for dp in range(2):      # head GEMM per patch (M=32, top 16 real)
                        p_ = 2 * j + dp
                        o_ps = psv.tile([32, BC], f32, tag="vps")
                        for hc in range(4):
                            nc.tensor.matmul(
                                o_ps[:, :],
                                wg2_s[:, hc * 32:(hc + 1) * 32],
                                h2s[hc][:, dp * 512:(dp + 1) * 512],
                                start=(hc == 0), stop=(hc == 3),
                            )
                        # drain into big [128, 16*BC]: partition 32*(p%4), col-block p//4
                        pr, pcb = 32 * (p_ % 4), (p_ // 4) * BC
                        nc.vector.tensor_scalar_add(
                            big[pr:pr + 32, pcb:pcb + BC], o_ps[:, :],
                            bg2_s[pr:pr + 32, 0:1])

            grouped("q")
            exchange("q")
            grouped("k")
            exchange("k")

            qs_big = pbig.tile([128, 16 * BC], f32, tag="qsbig")
            ks_big = pbig.tile([128, 16 * BC], f32, tag="ksbig")
            global_stream("q", qs_big)
            global_stream("k", ks_big)

            # ============ dot product + logits ============
            prod = ks_big   # in-place q*k
            nc.vector.tensor_mul(prod[:, :], qs_big[:, :], ks_big[:, :])
            red = pmisc.tile([128, BC], fr, tag="red")
            with nc.allow_low_precision(reason="fp32r reduce of 8 fp32 blocks"):
                nc.vector.tensor_reduce(
                    red[:, :],
                    prod[:, :].rearrange("a (c b) -> a b c", b=BC),
                    axis=mybir.AxisListType.X,
                    op=mybir.AluOpType.add,
                )
            at_ps = psv.tile([1, BC], f32, tag="vps")
            nc.tensor.matmul(at_ps[0:1, :], one_s[:, 0:1], red[:, :],
                             start=True, stop=True)
            at_s = pmisc.tile([1, BC], f32, tag="at")
            nc.vector.tensor_copy(at_s[0:1, :], at_ps[0:1, :])
            nc.sync.dma_start(out[0:1, :], at_s[0:1, :])

    nc.compile()
    return nc


_NC_CACHE = None


def _get_nc():
    global _NC_CACHE
    if _NC_CACHE is None:
        _NC_CACHE = _build_nc()
    return _NC_CACHE


def _prep_inputs(q, k, W1q, b1q, W2q, b2q, W1k, b1k, W2k, b2k, Wg1, bg1, Wg2, bg2):
    f16 = np.float16
    f32c = lambda a: np.ascontiguousarray(a, dtype=np.float32)

    def pack_x(x):  # [B, 4096] -> per-core view [512, B] fp16 (feature-major)
        xT = np.ascontiguousarray(np.asarray(x, np.float32).astype(f16).T)
        return [xT[c * 512:(c + 1) * 512, :] for c in range(NCORES)]

    def pack_w1(W1):  # [G, 64, 512] -> per-core [512, 512] fp16
        w = np.asarray(W1, np.float32).astype(f16).reshape(G * D, H)
        return [w[c * GL * D:(c + 1) * GL * D, :] for c in range(NCORES)]

    def pack_w2(W2):  # [G, 512, 64] -> per-core [GL*128, 256] fp16
        w = np.asarray(W2, np.float32).astype(f16).reshape(G, 4, 128, 64)
        w = np.ascontiguousarray(w.transpose(0, 2, 1, 3)).reshape(G * 128, 256)
        return [w[c * GL * 128:(c + 1) * GL * 128, :] for c in range(NCORES)]

    def pack_b1(b1):  # [G, 512] -> per-core [128, GL*4] fp32
        w = np.asarray(b1, np.float32).reshape(G, 4, 128).transpose(2, 0, 1)
        w = np.ascontiguousarray(w).reshape(128, G * 4)
        return [w[:, c * GL * 4:(c + 1) * GL * 4] for c in range(NCORES)]

    def pack_b2(b2):  # [G, 64] -> per-core [64, GL] fp32
        w = f32c(np.asarray(b2, np.float32).T)
        return [w[:, c * GL:(c + 1) * GL] for c in range(NCORES)]

    xq_s = pack_x(q)
    xk_s = pack_x(k)
    w1q_s = pack_w1(W1q)
    w1k_s = pack_w1(W1k)
    w2q_s = pack_w2(W2q)
    w2k_s = pack_w2(W2k)
    b1q_s = pack_b1(b1q)
    b1k_s = pack_b1(b1k)
    b2q_s = pack_b2(b2q)
    b2k_s = pack_b2(b2k)

    wg1_p = np.concatenate([Wg1, Wg1], axis=0).astype(f16)      # [128, 512]
    wg2_p = np.zeros((128, 4, 32), dtype=f16)
    wg2_p[:, :, :E] = np.asarray(Wg2, np.float32).reshape(4, 128, E).transpose(1, 0, 2)
    wg2_p = wg2_p.reshape(128, 4 * 32)                          # [r, hc*32+e]
    bg1_p = f32c(np.asarray(bg1, np.float32).reshape(4, 128).T)  # [128, 4]
    bg2_p = np.zeros((4, 32), dtype=np.float32)
    bg2_p[:, :E] = np.asarray(bg2, np.float32)
    bg2_p = bg2_p.reshape(128, 1)
    ones_p = np.ones((128, 1), dtype=np.float32)

    in_maps = []
    for c in range(NCORES):
        in_maps.append({
            "xq": xq_s[c], "xk": xk_s[c],
            "w1q": w1q_s[c], "w1k": w1k_s[c],
            "w2q": w2q_s[c], "w2k": w2k_s[c],
            "b1q": b1q_s[c], "b1k": b1k_s[c],
            "b2q": b2q_s[c], "b2k": b2k_s[c],
            "wg1": wg1_p, "wg2": wg2_p,
            "bg1p": bg1_p, "bg2r": bg2_p, "ones128": ones_p,
        })
    return in_maps


def kernel(q, k, W1q, b1q, W2q, b2q, W1k, b1k, W2k, b2k, Wg1, bg1, Wg2, bg2,
           _trace=False, _tracedir=None):
    from concourse.bass_utils import run_bass_kernel_spmd

    in_maps = _prep_inputs(q, k, W1q, b1q, W2q, b2q, W1k, b1k, W2k, b2k,
                           Wg1, bg1, Wg2, bg2)
    nc = _get_nc()
    kw = {}
    if _trace:
        kw = dict(trace=True, tmpdir=_tracedir)
    res = run_bass_kernel_spmd(nc, in_maps, core_ids=list(range(NCORES)), **kw)
    logits = np.concatenate([res.results[c]["out"].reshape(BC)
                             for c in range(NCORES)]).astype(np.float64)
    m = logits.max()
    e = np.exp(logits - m)
    sm = (e / e.sum()).astype(np.float32)
    if _trace:
        kernel._last_trace = res
    return sm
